# revision 56
# baseline (speedup 1.0000x reference)
"""Additive fast-weight (linear attention) layer on 8 TRN2 NeuronCores.

Strategy:
  - Shard batch (B=8) across the 8 cores: the recurrent state and scan are
    independent per (b, h); slow-net / out-linear weights are replicated.
  - Per core: LayerNorm -> qkv matmul -> elu+1 feature map with sum-norm ->
    chunked-parallel linear attention (chunk C=128 converts the 2048-step
    scan into matmuls: O = Q.W0 + tril(Q K^T).V ; W0 += K^T V) -> out matmul
    -> residual.
  - Matmuls run in bf16 with fp32 PSUM accumulation; the recurrent state is
    accumulated in fp32 in SBUF.  LayerNorm gamma is folded into W_qkv and
    beta into a per-feature bias (added via a K=1 matmul) on the host.
"""

import os
import numpy as np
import ml_dtypes

import concourse.bass as bass
import concourse.mybir as mybir
import concourse.tile as tile
from concourse.bass_utils import run_bass_kernel_spmd
from concourse.masks import make_identity

F32 = mybir.dt.float32
BF16 = mybir.dt.bfloat16
AF = mybir.ActivationFunctionType
ALU = mybir.AluOpType

NUM_HEAD, DIM_HEAD, IN_DIM = 16, 64, 1024
SLEN, BSZ = 2048, 8
C = 128  # chunk length (tokens per chunk)
NPAIR = NUM_HEAD // 2  # head pairs (2 heads share a 128-partition tile)


# Max semaphore waits each ISA struct can encode (observed from walrus
# codegen failures / successes).  Excess waits are moved onto standalone
# EventSemaphore instructions on the same engine right before, which is
# sound for compute engines (sequencer executes in program order).
_WAIT_CAPS = {}
_DMA_TYPES = {"InstDMACopy", "InstDmaTransposeAnt"}
_NO_SPLIT = {
    "InstEventSemaphore", "InstCall", "InstUnconditionalBranch",
    "InstRegisterMove", "InstISA",
}


def _prune_implied_waits(nc):
    """Transitive wait reduction via vector clocks.

    Tile's sem assignment is per-proc minimal but NOT transitively minimal:
    it re-emits waits already implied by another wait's producer.  Walrus can
    encode only ~1 wait per instruction, so compute, per instruction, the
    vector clock implied by each wait and drop waits another wait implies.
    Streams: compute engines execute in order; DMA queues (identified by
    their queue semaphore) complete in FIFO order.
    """
    insts = []
    for f in nc.m.functions:
        for blk in f.blocks:
            insts.extend(blk.instructions)

    QUEUE_PREFIXES = ("DMAHW", "DMASW")
    # Only reason about monotonically-increasing semaphores (engine ticks and
    # DMA queue sems).  Barrier/block/stage sems are inc/dec cycles -> the
    # threshold-history bisect below is invalid for them.
    MONO_PREFIXES = ("DMAHW", "DMASW", "Pool_", "DVE_", "PE_", "Activation_", "SP_")

    def _mono(name):
        return any(name.startswith(p) for p in MONO_PREFIXES)

    def stream_of(inst):
        si = inst.sync_info
        if si is not None:
            for u in (si.on_update or []):
                if any(u.ant_name.startswith(p) for p in QUEUE_PREFIXES):
                    return ("q", u.ant_name)
        return ("e", str(inst.engine))

    stream_vc = {}          # stream -> dict sem->val (clock after last inst)
    sem_val = {}            # sem name -> current value
    sem_hist = {}           # sem name -> list of (value_after, vc dict)
    n_pruned = 0

    def vc_at(sem, val):
        # clock of the instruction whose update first made sem >= val
        if not _mono(sem):
            return None
        hist = sem_hist.get(sem)
        if not hist:
            return None
        import bisect
        i = bisect.bisect_left([h[0] for h in hist], val)
        if i >= len(hist):
            return None
        return hist[i][1]

    for inst in insts:
        si = inst.sync_info
        waits = list(si.on_wait) if si is not None and si.on_wait else []
        st = stream_of(inst)
        # GpSimd dispatches to 8 Q7 cores: engine program order does NOT
        # imply completion order, so never accumulate a Pool engine stream.
        pool_engine = st[0] == "e" and "Pool" in st[1]
        vc = {} if pool_engine else dict(stream_vc.get(st, {}))
        wvcs = []
        for w in waits:
            pvc = vc_at(w.ant_name, w.wait_value) or {}
            wvcs.append(pvc)
        # prune: wait i implied if stream vc or another wait's vc covers it
        own_q = st[1] if st[0] == "q" else None
        keep = []
        for i, w in enumerate(waits):
            if not _mono(w.ant_name) or w.ant_name == own_q:
                # barrier sems and same-queue ring flow control: keep as-is
                keep.append(w)
                continue
            if vc.get(w.ant_name, 0) >= w.wait_value:
                n_pruned += 1
                continue
            implied = False
            for j, w2 in enumerate(waits):
                if i == j:
                    continue
                if wvcs[j].get(w.ant_name, 0) >= w.wait_value and (
                        j < i or not (wvcs[i].get(w2.ant_name, 0) >= w2.wait_value)):
                    implied = True
                    break
            if implied:
                n_pruned += 1
            else:
                keep.append(w)
        if si is not None and len(keep) != len(waits):
            inst.sync_info = mybir.SyncInfo(on_wait=keep,
                                            on_update=list(si.on_update))
        # Dispatch-closure: the stream VC may only accumulate WAITS (a later
        # instruction dispatches after an earlier one's wait held; monotonic
        # sems keep holding).  An instruction's own updates fire at
        # RETIREMENT, which dispatch order does not imply -- they go only
        # into sem_hist (observers via a sem wait have proof of retirement).
        for pvc in wvcs:
            for k, v in pvc.items():
                if vc.get(k, 0) < v:
                    vc[k] = v
        for w in waits:
            if vc.get(w.ant_name, 0) < w.wait_value:
                vc[w.ant_name] = w.wait_value
        if si is not None:
            for u in (si.on_update or []):
                if u.update_mode in ("sem-inc", "sem-add-imm"):
                    nv = sem_val.get(u.ant_name, 0) + u.update_value
                elif u.update_mode in ("sem-dec", "sem-sub-imm"):
                    nv = sem_val.get(u.ant_name, 0) - u.update_value
                else:
                    nv = sem_val.get(u.ant_name, 0)
                sem_val[u.ant_name] = nv
                retire_vc = dict(vc)
                if retire_vc.get(u.ant_name, 0) < nv:
                    retire_vc[u.ant_name] = nv
                sem_hist.setdefault(u.ant_name, []).append((nv, retire_vc))
        if not pool_engine:
            stream_vc[st] = vc
    return n_pruned


def _split_excess_waits(nc):
    """Walrus can encode only one sem-wait per instruction (per ISA struct).

    Compute engines: move excess waits onto standalone EventSemaphore
    instructions on the same engine right before (program order makes this
    sound).  DMA descriptors: the wait fires the descriptor, so instead a
    chain of EventSemaphores on SP absorbs ALL original waits and then
    increments a fresh one-shot semaphore that the DMA's single slot waits on.
    """
    used_ids = set()
    for f in nc.m.functions:
        for blk in f.blocks:
            for inst in blk.instructions:
                si = inst.sync_info
                if si is not None:
                    for w in (si.on_wait or []):
                        if w.sync_type == 'semaphore':
                            used_ids.add(w.id)
                    for u in (si.on_update or []):
                        if u.sync_type == 'semaphore':
                            used_ids.add(u.id)
    free_ids = sorted([i for i in range(256) if i not in used_ids], reverse=True)  # pop() -> lowest first
    sink_id = free_ids.pop()

    def _sink_upd():
        return [mybir.SyncUpdate(sync_type='semaphore', id=sink_id,
                                 ant_name='wsink', update_mode='sem-inc',
                                 update_value=1, update_reg=None)]

    n_split = 0
    n_join = 0
    for f in nc.m.functions:
        for blk in f.blocks:
            out = []
            changed = False
            for inst in blk.instructions:
                si = inst.sync_info
                tn = type(inst).__name__
                if si is not None and si.on_wait and tn not in _NO_SPLIT:
                    waits = list(si.on_wait)
                    cap = _WAIT_CAPS.get(tn, 1)
                    is_dma = tn in _DMA_TYPES and inst.engine != mybir.EngineType.Pool
                    if is_dma and len(waits) > 1 and not os.environ.get('ALLOW_JOIN'):
                        raise RuntimeError(
                            f"multi-wait DMA {inst.name} {tn}: "
                            + str([f'{w.ant_name}>={w.wait_value}' for w in waits]))
                    if is_dma and len(waits) > 1 and os.environ.get('BISECT_DROP'):
                        inst.sync_info = mybir.SyncInfo(
                            on_wait=[waits[0]], on_update=list(si.on_update))
                        changed = True
                    elif is_dma and len(waits) > 1 and os.environ.get('ALLOW_JOIN'):
                        sem_id = free_ids.pop()
                        sem_name = f"wjoin{n_join}"
                        n_join += 1
                        for i, w in enumerate(waits):
                            ev = mybir.InstEventSemaphore(
                                name=f"I-wjoin-{n_join}-{i}", ins=[], outs=[])
                            ev.engine = mybir.EngineType.Pool
                            upd = _sink_upd()
                            if i == len(waits) - 1:
                                upd = [mybir.SyncUpdate(
                                    sync_type='semaphore', id=sem_id,
                                    ant_name=sem_name, update_mode='sem-inc',
                                    update_value=1, update_reg=None)]
                            ev.sync_info = mybir.SyncInfo(on_wait=[w], on_update=upd)
                            out.append(ev)
                        inst.sync_info = mybir.SyncInfo(
                            on_wait=[mybir.SyncWait(
                                sync_type='semaphore', id=sem_id,
                                ant_name=sem_name, wait_mode='sem-ge-imm',
                                wait_value=1, wait_reg=None)],
                            on_update=list(si.on_update))
                        changed = True
                    elif not is_dma and len(waits) > cap:
                        for w in waits[cap:]:
                            ev = mybir.InstEventSemaphore(
                                name=f"I-wsplit-{n_split}", ins=[], outs=[])
                            n_split += 1
                            ev.engine = inst.engine
                            ev.sync_info = mybir.SyncInfo(on_wait=[w], on_update=_sink_upd())
                            out.append(ev)
                        inst.sync_info = mybir.SyncInfo(
                            on_wait=waits[:cap], on_update=list(si.on_update))
                        changed = True
                out.append(inst)
            if changed:
                blk.instructions = out
    if n_join or n_split:
        reg = dict(nc.m.ant_sem_names)
        reg[str(sink_id)] = ['wsink']
        for f in nc.m.functions:
            for blk in f.blocks:
                for inst in blk.instructions:
                    si = inst.sync_info
                    if si is not None:
                        for u in (si.on_update or []):
                            if u.ant_name.startswith("wjoin"):
                                reg[str(u.id)] = [u.ant_name]
        nc.m.ant_sem_names = reg
    return n_split + n_join


def build_nc(n_chunks: int = SLEN // C, split_waits: bool = True) -> bass.Bass:
    S = n_chunks * C
    nc = bass.Bass()

    x_in = nc.declare_dram_parameter("x", [S, IN_DIM], F32, isOutput=False)
    st_in = nc.declare_dram_parameter("st", [128, NPAIR, 128], F32, isOutput=False)
    wq_in = nc.declare_dram_parameter("wq", [IN_DIM, IN_DIM], BF16, isOutput=False)
    wk_in = nc.declare_dram_parameter("wk", [IN_DIM, IN_DIM], BF16, isOutput=False)
    wv_in = nc.declare_dram_parameter("wv", [IN_DIM, IN_DIM], BF16, isOutput=False)
    wo_in = nc.declare_dram_parameter("wo", [IN_DIM, IN_DIM], BF16, isOutput=False)
    bias_in = nc.declare_dram_parameter("bqkv", [1, 3 * IN_DIM], BF16, isOutput=False)
    mask_in = nc.declare_dram_parameter("mask", [128, 128], F32, isOutput=False)
    y_out = nc.declare_dram_parameter("y", [S, IN_DIM], F32, isOutput=True)
    wf_out = nc.declare_dram_parameter("wf", [128, NPAIR, 128], F32, isOutput=True)

    xv = x_in.rearrange("(n c) i -> n c i", c=C)
    yv = y_out.rearrange("(n c) i -> n c i", c=C)

    with tile.TileContext(nc) as tc:
        with (
            tc.tile_pool(name="singles", bufs=1) as singles,
            tc.tile_pool(name="work", bufs=2) as work,
            tc.tile_pool(name="small", bufs=3) as small,
            tc.tile_pool(name="ampool", bufs=4) as ampool,
            tc.tile_pool(name="qkv_ps", bufs=3, space="PSUM") as qkv_ps,
            tc.tile_pool(name="attn_ps", bufs=3, space="PSUM") as attn_ps,
            tc.tile_pool(name="tp_ps", bufs=2, space="PSUM") as tp_ps,
        ):
            # ---- persistent tiles ----
            wq_sb = singles.tile([128, 8, IN_DIM], BF16, tag="wq")
            wk_sb = singles.tile([128, 8, IN_DIM], BF16, tag="wk")
            wv_sb = singles.tile([128, 8, IN_DIM], BF16, tag="wv")
            wo_sb = singles.tile([128, 8, IN_DIM], BF16, tag="wo")
            for sb, src in ((wq_sb, wq_in), (wk_sb, wk_in), (wv_sb, wv_in), (wo_sb, wo_in)):
                nc.gpsimd.dma_start(out=sb, in_=src.rearrange("(ko ki) n -> ki ko n", ki=128))

            bias_sb = singles.tile([1, 3 * IN_DIM], BF16, tag="bias")
            nc.gpsimd.dma_start(out=bias_sb, in_=bias_in[:])
            ones_sb = singles.tile([1, 128], BF16, tag="ones")
            nc.vector.memset(ones_sb, 1.0)
            mask_sb = singles.tile([128, 128], F32, tag="mask")
            nc.gpsimd.dma_start(out=mask_sb, in_=mask_in[:])
            eps_sb = singles.tile([128, 1], F32, tag="eps")
            nc.vector.memset(eps_sb, 1e-5)
            ident = singles.tile([128, 128], BF16, tag="ident")
            make_identity(nc, ident)

            def pe_transpose(dst, src, n_tiles):
                # src: SBUF bf16 [128, n*128]; dst: SBUF bf16 [128, n, 128]
                for g in range(0, n_tiles, 4):
                    gn = min(4, n_tiles - g)
                    tp = tp_ps.tile([128, 4, 128], BF16, tag="tp")
                    for j in range(gn):
                        nc.tensor.transpose(tp[:, j], src[:, 128 * (g + j): 128 * (g + j + 1)], ident)
                    nc.scalar.copy(out=dst[:, g:g + gn], in_=tp[:, :gn])

            # recurrent state, fp32 accumulator + bf16 mirror for matmul
            state_sb = singles.tile([128, NPAIR, 128], F32, tag="state")
            nc.gpsimd.dma_start(out=state_sb, in_=st_in[:])
            state_bf = singles.tile([128, NPAIR, 128], BF16, tag="statebf")
            nc.vector.tensor_copy(out=state_bf, in_=state_sb)
            touch = singles.tile([1, 8], F32, tag="touch")
            for ti, tsrc in enumerate((wq_sb, wk_sb, wv_sb, wo_sb, bias_sb, mask_sb)):
                nc.vector.tensor_copy(out=touch[0:1, ti:ti + 1].bitcast(tsrc.dtype)[:, 0:1],
                                      in_=tsrc[0:1, 0:1] if len(tsrc.shape) == 2 else tsrc[0:1, 0, 0:1])
            # y accumulates on-chip (bf16); single casting DMA at the end
            y_big = singles.tile([128, n_chunks, IN_DIM], BF16, tag="ybig")

            # stage ALL of x on-chip in bf16 up front (gpsimd DMA casts):
            # during the chunk loop no plain DMA is in flight, so the
            # DMA-transposes never pick up xbar-serialization waits.
            x_all = singles.tile([128, n_chunks, IN_DIM], BF16, tag="xall")
            xcv = x_in.rearrange("(n c) i -> c n i", c=C)
            n_xdma = 4 if n_chunks % 4 == 0 else 1
            xg = n_chunks // n_xdma
            for g in range(n_xdma):
                nc.gpsimd.dma_start(out=x_all[:, g * xg:(g + 1) * xg],
                                    in_=xcv[:, g * xg:(g + 1) * xg])
            for g in range(n_xdma):
                # value-preserving dependency: chunk-0 LN now transitively
                # carries every x-staging DMA in its DVE wait closure
                nc.vector.tensor_tensor(out=x_all[0:1, 0, 0:1], in0=x_all[0:1, 0, 0:1],
                                        in1=x_all[0:1, g * xg, 0:1], op=ALU.bypass)

            prev_tdst = [None]

            def dma_T(dst, src):
                # DMA transposes share XBAR state: Tile serializes them across
                # queues.  Route that ordering through the DVE stream (bypass
                # dep on the previous transpose's dst) + full-tile memset so
                # the transpose itself needs only its single DVE wait.
                if prev_tdst[0] is not None:
                    nc.vector.tensor_tensor(out=dst[0:1, 0, 0:1], in0=dst[0:1, 0, 0:1],
                                            in1=prev_tdst[0][0:1, 0, 0:1], op=ALU.bypass)
                nc.vector.memset(dst, 0.0)
                nc.sync.dma_start_transpose(dst, src)
                prev_tdst[0] = dst

            for c in range(n_chunks):
                x_t = x_all[:, c]

                # ---- LayerNorm (gamma/beta folded into weights on host) ----
                stats = small.tile([128, 2, 6], F32, tag="stats")
                nc.vector.bn_stats(out=stats[:, 0], in_=x_t[:, 0:512])
                nc.vector.bn_stats(out=stats[:, 1], in_=x_t[:, 512:1024])
                mv = small.tile([128, 2], F32, tag="mv")
                nc.vector.bn_aggr(out=mv, in_=stats)
                rstd = small.tile([128, 1], F32, tag="rstd")
                nc.scalar.activation(out=rstd, in_=mv[:, 1:2], func=AF.Sqrt, bias=eps_sb)
                nc.vector.reciprocal(out=rstd, in_=rstd)
                h_t = work.tile([128, IN_DIM], BF16, tag="h")
                nc.vector.tensor_scalar(out=h_t, in0=x_t, scalar1=mv[:, 0:1],
                                        scalar2=rstd, op0=ALU.subtract, op1=ALU.mult)

                # ---- transpose h (feature-major for matmul contraction) ----
                hT = work.tile([128, 8, 128], BF16, tag="hT")
                dma_T(hT, h_t)

                # ---- qkv matmuls (natural layout: tokens on partitions) ----
                def qkv_matmul_half(w_sb, bias_off, n):
                    ps = qkv_ps.tile([128, 512], F32, tag="qkvps", name="qkvps")
                    sl = slice(n * 512, (n + 1) * 512)
                    nc.tensor.matmul(ps, lhsT=ones_sb,
                                     rhs=bias_sb[:, bias_off + n * 512: bias_off + (n + 1) * 512],
                                     start=True, stop=False)
                    for ki in range(8):
                        nc.tensor.matmul(ps, lhsT=hT[:, ki], rhs=w_sb[:, ki, sl],
                                         start=False, stop=(ki == 7))
                    return ps

                q_ps = [qkv_matmul_half(wq_sb, 0, n) for n in range(2)]
                k_ps = [qkv_matmul_half(wk_sb, IN_DIM, n) for n in range(2)]
                v_ps = [qkv_matmul_half(wv_sb, 2 * IN_DIM, n) for n in range(2)]

                # ---- feature map: elu(x)+1 = min(exp(x),1) + relu(x), then /(sum+eps)
                def elu_norm(ps_halves, tag):
                    nrm = work.tile([128, NUM_HEAD, DIM_HEAD], BF16, tag="n" + tag)
                    for n, ps in enumerate(ps_halves):
                        hh = slice(8 * n, 8 * n + 8)  # heads in this half
                        ex = work.tile([128, 512], F32, tag="ex", name="ex")
                        nc.scalar.activation(out=ex, in_=ps, func=AF.Exp)
                        rl = work.tile([128, 512], F32, tag="rl", name="rl")
                        nc.scalar.activation(out=rl, in_=ps, func=AF.Relu)
                        el = work.tile([128, 8, DIM_HEAD], F32, tag="el", name="el")
                        nc.vector.scalar_tensor_tensor(out=el.rearrange("p h d -> p (h d)"),
                                                       in0=ex, scalar=1.0,
                                                       in1=rl, op0=ALU.min, op1=ALU.add)
                        sm = small.tile([128, 8], F32, tag="s" + tag, name="sm")
                        nc.vector.tensor_reduce(out=sm, in_=el, axis=mybir.AxisListType.X, op=ALU.add)
                        nc.vector.tensor_scalar_add(out=sm, in0=sm, scalar1=1e-5)
                        nc.vector.reciprocal(out=sm, in_=sm)
                        nc.vector.tensor_tensor(out=nrm[:, hh], in0=el,
                                                in1=sm[:, :, None].to_broadcast(el.shape),
                                                op=ALU.mult)
                    return nrm

                qn = elu_norm(q_ps, "q")
                kn = elu_norm(k_ps, "k")
                vn = work.tile([128, NUM_HEAD, DIM_HEAD], BF16, tag="vn")
                for n in range(2):
                    nc.scalar.copy(out=vn[:, 8 * n: 8 * n + 8].rearrange("p h d -> p (h d)"),
                                   in_=v_ps[n])

                # ---- transpose q,k (head-dim on partitions) ----
                qnT = work.tile([128, NPAIR, 128], BF16, tag="qnT")
                knT = work.tile([128, NPAIR, 128], BF16, tag="knT")
                dma_T(qnT, qn.rearrange("p h d -> p (h d)"))
                dma_T(knT, kn.rearrange("p h d -> p (h d)"))

                # ---- per-head chunked attention ----
                oT = work.tile([128, NPAIR, 128], BF16, tag="oT")
                for h in range(NUM_HEAD):
                    p, half = h // 2, h % 2
                    prt = slice(64 * half, 64 * half + 64)
                    at = attn_ps.tile([128, 3, 128], F32, tag="at")
                    nc.tensor.matmul(at[:, 0], lhsT=knT[prt, p], rhs=qnT[prt, p],
                                     start=True, stop=True)
                    am = ampool.tile([128, 128], BF16, tag="am")
                    nc.vector.tensor_tensor(out=am, in0=at[:, 0], in1=mask_sb, op=ALU.mult)
                    nc.tensor.matmul(at[0:64, 1], lhsT=state_bf[prt, p, prt],
                                     rhs=qnT[prt, p], start=True, stop=False)
                    nc.tensor.matmul(at[0:64, 1], lhsT=vn[:, h], rhs=am,
                                     start=False, stop=True)
                    nc.scalar.copy(out=oT[prt, p], in_=at[0:64, 1])
                    nc.tensor.matmul(at[0:64, 2, 0:64], lhsT=kn[:, h], rhs=vn[:, h],
                                     start=True, stop=True)
                    nc.vector.tensor_add(out=state_sb[prt, p, prt],
                                         in0=state_sb[prt, p, prt], in1=at[0:64, 2, 0:64])
                    nc.scalar.copy(out=state_bf[prt, p, prt], in_=state_sb[prt, p, prt])

                # ---- out matmul + residual ----
                for n in range(2):
                    sl = slice(n * 512, (n + 1) * 512)
                    ops = qkv_ps.tile([128, 512], F32, tag="qkvps", name="ops")
                    for p8 in range(8):
                        nc.tensor.matmul(ops, lhsT=oT[:, p8], rhs=wo_sb[:, p8, sl],
                                         start=(p8 == 0), stop=(p8 == 7))
                    nc.vector.tensor_add(out=y_big[:, c, sl], in0=ops, in1=x_t[:, sl])

            # ---- final outputs ----
            nc.gpsimd.dma_start(out=y_out.rearrange("(n c) i -> c n i", c=C), in_=y_big)
            nc.gpsimd.dma_start(out=wf_out[:], in_=state_sb)

    if split_waits:
        if not os.environ.get('NO_PRUNE'):
            _prune_implied_waits(nc)
        _split_excess_waits(nc)
    return nc


# ---------------- host side ----------------

def _prep_weights(W_qkv, W_out, ln_gamma, ln_beta):
    H, D3 = NUM_HEAD, 3 * DIM_HEAD
    W_eff = (W_qkv * ln_gamma[None, :]).astype(np.float32)
    bias_full = W_qkv.astype(np.float64) @ ln_beta.astype(np.float64)  # [3072]
    Wr = W_eff.reshape(H, D3, IN_DIM)
    bq = bias_full.reshape(H, D3)
    wq = Wr[:, 0:DIM_HEAD].reshape(H * DIM_HEAD, IN_DIM)
    wk = Wr[:, DIM_HEAD:2 * DIM_HEAD].reshape(H * DIM_HEAD, IN_DIM)
    wv = Wr[:, 2 * DIM_HEAD:].reshape(H * DIM_HEAD, IN_DIM)
    bias = np.concatenate([bq[:, 0:DIM_HEAD].reshape(-1),
                           bq[:, DIM_HEAD:2 * DIM_HEAD].reshape(-1),
                           bq[:, 2 * DIM_HEAD:].reshape(-1)])[None, :]
    bf = ml_dtypes.bfloat16
    return (np.ascontiguousarray(wq.T).astype(bf),
            np.ascontiguousarray(wk.T).astype(bf),
            np.ascontiguousarray(wv.T).astype(bf),
            np.ascontiguousarray(W_out.T).astype(bf),
            bias.astype(bf))


def _pack_state(state_b):
    out = np.zeros((128, NPAIR, 128), np.float32)
    for p in range(NPAIR):
        out[0:64, p, 0:64] = state_b[2 * p]
        out[64:128, p, 64:128] = state_b[2 * p + 1]
    return out


def _unpack_state(wf_dev):
    out = np.empty((NUM_HEAD, DIM_HEAD, DIM_HEAD), np.float32)
    for p in range(NPAIR):
        out[2 * p] = wf_dev[0:64, p, 0:64]
        out[2 * p + 1] = wf_dev[64:128, p, 64:128]
    return out


_NC_CACHE = {}
TRACE = False
LAST_RESULT = None


def kernel(x, state, W_qkv, W_out, ln_gamma, ln_beta):
    x = np.asarray(x, np.float32)
    state = np.asarray(state, np.float32)
    S, B, _ = x.shape
    n_chunks = S // C
    if n_chunks not in _NC_CACHE:
        _NC_CACHE[n_chunks] = build_nc(n_chunks)
    nc = _NC_CACHE[n_chunks]

    wq, wk, wv, wo, bias = _prep_weights(np.asarray(W_qkv, np.float32),
                                         np.asarray(W_out, np.float32),
                                         np.asarray(ln_gamma, np.float32),
                                         np.asarray(ln_beta, np.float32))
    mask = np.triu(np.ones((128, 128), np.float32))

    in_maps = []
    for b in range(B):
        in_maps.append({
            "x": np.ascontiguousarray(x[:, b, :]),
            "st": _pack_state(state[b]),
            "wq": wq, "wk": wk, "wv": wv, "wo": wo,
            "bqkv": bias, "mask": mask,
        })
    global LAST_RESULT
    res = run_bass_kernel_spmd(nc, in_maps, core_ids=list(range(B)), trace=TRACE)
    LAST_RESULT = res

    y = np.empty((S, B, IN_DIM), np.float32)
    wf = np.empty((B, NUM_HEAD, DIM_HEAD, DIM_HEAD), np.float32)
    for b in range(B):
        y[:, b, :] = res.results[b]["y"]
        wf[b] = _unpack_state(res.results[b]["wf"])
    return y, wf


# revision 58
# speedup vs baseline: 1.1569x; 1.1569x over previous
"""Additive fast-weight (linear attention) layer on 8 TRN2 NeuronCores.

Strategy:
  - Shard batch (B=8) across the 8 cores: the recurrent state and scan are
    independent per (b, h); slow-net / out-linear weights are replicated.
  - Per core: LayerNorm -> qkv matmul -> elu+1 feature map with sum-norm ->
    chunked-parallel linear attention (chunk C=128 converts the 2048-step
    scan into matmuls: O = Q.W0 + tril(Q K^T).V ; W0 += K^T V) -> out matmul
    -> residual.
  - Matmuls run in bf16 with fp32 PSUM accumulation; the recurrent state is
    accumulated in fp32 in SBUF.  LayerNorm gamma is folded into W_qkv and
    beta into a per-feature bias (added via a K=1 matmul) on the host.
"""

import os
import numpy as np
import ml_dtypes

import concourse.bass as bass
import concourse.mybir as mybir
import concourse.tile as tile
from concourse.bass_utils import run_bass_kernel_spmd
from concourse.masks import make_identity

F32 = mybir.dt.float32
BF16 = mybir.dt.bfloat16
AF = mybir.ActivationFunctionType
ALU = mybir.AluOpType

NUM_HEAD, DIM_HEAD, IN_DIM = 16, 64, 1024
SLEN, BSZ = 2048, 8
C = 128  # chunk length (tokens per chunk)
NPAIR = NUM_HEAD // 2  # head pairs (2 heads share a 128-partition tile)


# Max semaphore waits each ISA struct can encode (observed from walrus
# codegen failures / successes).  Excess waits are moved onto standalone
# EventSemaphore instructions on the same engine right before, which is
# sound for compute engines (sequencer executes in program order).
_WAIT_CAPS = {}
_DMA_TYPES = {"InstDMACopy", "InstDmaTransposeAnt"}
_NO_SPLIT = {
    "InstEventSemaphore", "InstCall", "InstUnconditionalBranch",
    "InstRegisterMove", "InstISA",
}


def _prune_implied_waits(nc):
    """Transitive wait reduction via vector clocks.

    Tile's sem assignment is per-proc minimal but NOT transitively minimal:
    it re-emits waits already implied by another wait's producer.  Walrus can
    encode only ~1 wait per instruction, so compute, per instruction, the
    vector clock implied by each wait and drop waits another wait implies.
    Streams: compute engines execute in order; DMA queues (identified by
    their queue semaphore) complete in FIFO order.
    """
    insts = []
    for f in nc.m.functions:
        for blk in f.blocks:
            insts.extend(blk.instructions)

    QUEUE_PREFIXES = ("DMAHW", "DMASW")
    # Only reason about monotonically-increasing semaphores (engine ticks and
    # DMA queue sems).  Barrier/block/stage sems are inc/dec cycles -> the
    # threshold-history bisect below is invalid for them.
    MONO_PREFIXES = ("DMAHW", "DMASW", "Pool_", "DVE_", "PE_", "Activation_", "SP_")

    def _mono(name):
        return any(name.startswith(p) for p in MONO_PREFIXES)

    def stream_of(inst):
        si = inst.sync_info
        if si is not None:
            for u in (si.on_update or []):
                if any(u.ant_name.startswith(p) for p in QUEUE_PREFIXES):
                    return ("q", u.ant_name)
        return ("e", str(inst.engine))

    stream_vc = {}          # stream -> dict sem->val (clock after last inst)
    sem_val = {}            # sem name -> current value
    sem_hist = {}           # sem name -> list of (value_after, vc dict)
    n_pruned = 0

    def vc_at(sem, val):
        # clock of the instruction whose update first made sem >= val
        if not _mono(sem):
            return None
        hist = sem_hist.get(sem)
        if not hist:
            return None
        import bisect
        i = bisect.bisect_left([h[0] for h in hist], val)
        if i >= len(hist):
            return None
        return hist[i][1]

    for inst in insts:
        si = inst.sync_info
        waits = list(si.on_wait) if si is not None and si.on_wait else []
        st = stream_of(inst)
        # GpSimd dispatches to 8 Q7 cores: engine program order does NOT
        # imply completion order, so never accumulate a Pool engine stream.
        pool_engine = st[0] == "e" and "Pool" in st[1]
        vc = {} if pool_engine else dict(stream_vc.get(st, {}))
        wvcs = []
        for w in waits:
            pvc = vc_at(w.ant_name, w.wait_value) or {}
            wvcs.append(pvc)
        # prune: wait i implied if stream vc or another wait's vc covers it
        own_q = st[1] if st[0] == "q" else None
        keep = []
        for i, w in enumerate(waits):
            if not _mono(w.ant_name) or w.ant_name == own_q:
                # barrier sems and same-queue ring flow control: keep as-is
                keep.append(w)
                continue
            if vc.get(w.ant_name, 0) >= w.wait_value:
                n_pruned += 1
                continue
            implied = False
            for j, w2 in enumerate(waits):
                if i == j:
                    continue
                if wvcs[j].get(w.ant_name, 0) >= w.wait_value and (
                        j < i or not (wvcs[i].get(w2.ant_name, 0) >= w2.wait_value)):
                    implied = True
                    break
            if implied:
                n_pruned += 1
            else:
                keep.append(w)
        if si is not None and len(keep) != len(waits):
            inst.sync_info = mybir.SyncInfo(on_wait=keep,
                                            on_update=list(si.on_update))
        # Dispatch-closure: the stream VC may only accumulate WAITS (a later
        # instruction dispatches after an earlier one's wait held; monotonic
        # sems keep holding).  An instruction's own updates fire at
        # RETIREMENT, which dispatch order does not imply -- they go only
        # into sem_hist (observers via a sem wait have proof of retirement).
        for pvc in wvcs:
            for k, v in pvc.items():
                if vc.get(k, 0) < v:
                    vc[k] = v
        for w in waits:
            if vc.get(w.ant_name, 0) < w.wait_value:
                vc[w.ant_name] = w.wait_value
        if si is not None:
            for u in (si.on_update or []):
                if u.update_mode in ("sem-inc", "sem-add-imm"):
                    nv = sem_val.get(u.ant_name, 0) + u.update_value
                elif u.update_mode in ("sem-dec", "sem-sub-imm"):
                    nv = sem_val.get(u.ant_name, 0) - u.update_value
                else:
                    nv = sem_val.get(u.ant_name, 0)
                sem_val[u.ant_name] = nv
                retire_vc = dict(vc)
                if retire_vc.get(u.ant_name, 0) < nv:
                    retire_vc[u.ant_name] = nv
                sem_hist.setdefault(u.ant_name, []).append((nv, retire_vc))
        if not pool_engine:
            stream_vc[st] = vc
    return n_pruned


def _split_excess_waits(nc):
    """Walrus can encode only one sem-wait per instruction (per ISA struct).

    Compute engines: move excess waits onto standalone EventSemaphore
    instructions on the same engine right before (program order makes this
    sound).  DMA descriptors: the wait fires the descriptor, so instead a
    chain of EventSemaphores on SP absorbs ALL original waits and then
    increments a fresh one-shot semaphore that the DMA's single slot waits on.
    """
    used_ids = set()
    for f in nc.m.functions:
        for blk in f.blocks:
            for inst in blk.instructions:
                si = inst.sync_info
                if si is not None:
                    for w in (si.on_wait or []):
                        if w.sync_type == 'semaphore':
                            used_ids.add(w.id)
                    for u in (si.on_update or []):
                        if u.sync_type == 'semaphore':
                            used_ids.add(u.id)
    free_ids = sorted([i for i in range(256) if i not in used_ids], reverse=True)  # pop() -> lowest first
    sink_id = free_ids.pop()

    def _sink_upd():
        return [mybir.SyncUpdate(sync_type='semaphore', id=sink_id,
                                 ant_name='wsink', update_mode='sem-inc',
                                 update_value=1, update_reg=None)]

    n_split = 0
    n_join = 0
    for f in nc.m.functions:
        for blk in f.blocks:
            out = []
            changed = False
            for inst in blk.instructions:
                si = inst.sync_info
                tn = type(inst).__name__
                if si is not None and si.on_wait and tn not in _NO_SPLIT:
                    waits = list(si.on_wait)
                    cap = _WAIT_CAPS.get(tn, 1)
                    is_dma = tn in _DMA_TYPES and inst.engine != mybir.EngineType.Pool
                    if is_dma and len(waits) > 1 and not os.environ.get('ALLOW_JOIN'):
                        raise RuntimeError(
                            f"multi-wait DMA {inst.name} {tn}: "
                            + str([f'{w.ant_name}>={w.wait_value}' for w in waits]))
                    if is_dma and len(waits) > 1 and os.environ.get('BISECT_DROP'):
                        inst.sync_info = mybir.SyncInfo(
                            on_wait=[waits[0]], on_update=list(si.on_update))
                        changed = True
                    elif is_dma and len(waits) > 1 and os.environ.get('ALLOW_JOIN'):
                        sem_id = free_ids.pop()
                        sem_name = f"wjoin{n_join}"
                        n_join += 1
                        for i, w in enumerate(waits):
                            ev = mybir.InstEventSemaphore(
                                name=f"I-wjoin-{n_join}-{i}", ins=[], outs=[])
                            ev.engine = mybir.EngineType.Pool
                            upd = _sink_upd()
                            if i == len(waits) - 1:
                                upd = [mybir.SyncUpdate(
                                    sync_type='semaphore', id=sem_id,
                                    ant_name=sem_name, update_mode='sem-inc',
                                    update_value=1, update_reg=None)]
                            ev.sync_info = mybir.SyncInfo(on_wait=[w], on_update=upd)
                            out.append(ev)
                        inst.sync_info = mybir.SyncInfo(
                            on_wait=[mybir.SyncWait(
                                sync_type='semaphore', id=sem_id,
                                ant_name=sem_name, wait_mode='sem-ge-imm',
                                wait_value=1, wait_reg=None)],
                            on_update=list(si.on_update))
                        changed = True
                    elif not is_dma and len(waits) > cap:
                        for w in waits[cap:]:
                            ev = mybir.InstEventSemaphore(
                                name=f"I-wsplit-{n_split}", ins=[], outs=[])
                            n_split += 1
                            ev.engine = inst.engine
                            ev.sync_info = mybir.SyncInfo(on_wait=[w], on_update=_sink_upd())
                            out.append(ev)
                        inst.sync_info = mybir.SyncInfo(
                            on_wait=waits[:cap], on_update=list(si.on_update))
                        changed = True
                out.append(inst)
            if changed:
                blk.instructions = out
    if n_join or n_split:
        reg = dict(nc.m.ant_sem_names)
        reg[str(sink_id)] = ['wsink']
        for f in nc.m.functions:
            for blk in f.blocks:
                for inst in blk.instructions:
                    si = inst.sync_info
                    if si is not None:
                        for u in (si.on_update or []):
                            if u.ant_name.startswith("wjoin"):
                                reg[str(u.id)] = [u.ant_name]
        nc.m.ant_sem_names = reg
    return n_split + n_join


def build_nc(n_chunks: int = SLEN // C, split_waits: bool = True) -> bass.Bass:
    S = n_chunks * C
    nc = bass.Bass()

    x_in = nc.declare_dram_parameter("x", [S, IN_DIM], F32, isOutput=False)
    st_in = nc.declare_dram_parameter("st", [128, NPAIR, 128], F32, isOutput=False)
    wq_in = nc.declare_dram_parameter("wq", [IN_DIM, IN_DIM], BF16, isOutput=False)
    wk_in = nc.declare_dram_parameter("wk", [IN_DIM, IN_DIM], BF16, isOutput=False)
    wv_in = nc.declare_dram_parameter("wv", [IN_DIM, IN_DIM], BF16, isOutput=False)
    wo_in = nc.declare_dram_parameter("wo", [IN_DIM, IN_DIM], BF16, isOutput=False)
    bias_in = nc.declare_dram_parameter("bqkv", [1, 3 * IN_DIM], BF16, isOutput=False)
    mask_in = nc.declare_dram_parameter("mask", [128, 128], F32, isOutput=False)
    y_out = nc.declare_dram_parameter("y", [S, IN_DIM], F32, isOutput=True)
    wf_out = nc.declare_dram_parameter("wf", [128, NPAIR, 128], F32, isOutput=True)

    xv = x_in.rearrange("(n c) i -> n c i", c=C)
    yv = y_out.rearrange("(n c) i -> n c i", c=C)

    with tile.TileContext(nc) as tc:
        with (
            tc.tile_pool(name="singles", bufs=1) as singles,
            tc.tile_pool(name="xy", bufs=3) as xy,
            tc.tile_pool(name="work", bufs=2) as work,
            tc.tile_pool(name="small", bufs=3) as small,
            tc.tile_pool(name="ampool", bufs=4) as ampool,
            tc.tile_pool(name="qkv_ps", bufs=2, space="PSUM") as qkv_ps,
            tc.tile_pool(name="attn_ps", bufs=2, space="PSUM") as attn_ps,
            tc.tile_pool(name="tp_ps", bufs=2, space="PSUM") as tp_ps,
        ):
            # ---- persistent tiles ----
            wq_sb = singles.tile([128, 8, IN_DIM], BF16, tag="wq")
            wk_sb = singles.tile([128, 8, IN_DIM], BF16, tag="wk")
            wv_sb = singles.tile([128, 8, IN_DIM], BF16, tag="wv")
            wo_sb = singles.tile([128, 8, IN_DIM], BF16, tag="wo")
            for sb, src in ((wq_sb, wq_in), (wk_sb, wk_in), (wv_sb, wv_in), (wo_sb, wo_in)):
                nc.gpsimd.dma_start(out=sb, in_=src.rearrange("(ko ki) n -> ki ko n", ki=128))

            bias_sb = singles.tile([1, 3 * IN_DIM], BF16, tag="bias")
            nc.gpsimd.dma_start(out=bias_sb, in_=bias_in[:])
            ones_sb = singles.tile([1, 128], BF16, tag="ones")
            nc.vector.memset(ones_sb, 1.0)
            mask_sb = singles.tile([128, 128], F32, tag="mask")
            nc.gpsimd.dma_start(out=mask_sb, in_=mask_in[:])
            eps_sb = singles.tile([128, 1], F32, tag="eps")
            nc.vector.memset(eps_sb, 1e-5)
            ident = singles.tile([128, 128], BF16, tag="ident")
            make_identity(nc, ident)

            def pe_transpose(dst, src, n_tiles):
                # src: SBUF bf16 [128, n*128]; dst: SBUF bf16 [128, n, 128]
                for g in range(0, n_tiles, 4):
                    gn = min(4, n_tiles - g)
                    tp = tp_ps.tile([128, 4, 128], BF16, tag="tp")
                    for j in range(gn):
                        nc.tensor.transpose(tp[:, j], src[:, 128 * (g + j): 128 * (g + j + 1)], ident)
                    nc.scalar.copy(out=dst[:, g:g + gn], in_=tp[:, :gn])

            # recurrent state, fp32 accumulator + bf16 mirror for matmul
            state_sb = singles.tile([128, NPAIR, 128], F32, tag="state")
            nc.gpsimd.dma_start(out=state_sb, in_=st_in[:])
            state_bf = singles.tile([128, NPAIR, 128], BF16, tag="statebf")
            nc.vector.tensor_copy(out=state_bf, in_=state_sb)
            touch = singles.tile([1, 8], F32, tag="touch")
            for ti, tsrc in enumerate((wq_sb, wk_sb, wv_sb, wo_sb, bias_sb, mask_sb)):
                nc.vector.tensor_copy(out=touch[0:1, ti:ti + 1].bitcast(tsrc.dtype)[:, 0:1],
                                      in_=tsrc[0:1, 0:1] if len(tsrc.shape) == 2 else tsrc[0:1, 0, 0:1])
            # y accumulates on-chip (bf16); single casting DMA at the end
            y_big = singles.tile([128, n_chunks, IN_DIM], BF16, tag="ybig")

            for c in range(n_chunks):
                # ---- load x chunk ----
                x_t = xy.tile([128, IN_DIM], F32, tag="x")
                nc.gpsimd.dma_start(out=x_t, in_=xv[c])

                # ---- LayerNorm (gamma/beta folded into weights on host) ----
                stats = small.tile([128, 2, 6], F32, tag="stats")
                nc.vector.bn_stats(out=stats[:, 0], in_=x_t[:, 0:512])
                nc.vector.bn_stats(out=stats[:, 1], in_=x_t[:, 512:1024])
                mv = small.tile([128, 2], F32, tag="mv")
                nc.vector.bn_aggr(out=mv, in_=stats)
                rstd = small.tile([128, 1], F32, tag="rstd")
                nc.scalar.activation(out=rstd, in_=mv[:, 1:2], func=AF.Sqrt, bias=eps_sb)
                nc.vector.reciprocal(out=rstd, in_=rstd)
                h_t = work.tile([128, IN_DIM], BF16, tag="h")
                nc.vector.tensor_scalar(out=h_t, in0=x_t, scalar1=mv[:, 0:1],
                                        scalar2=rstd, op0=ALU.subtract, op1=ALU.mult)

                # ---- transpose h (feature-major for matmul contraction) ----
                hT = work.tile([128, 8, 128], BF16, tag="hT")
                pe_transpose(hT, h_t, 8)

                # ---- qkv matmuls (natural layout: tokens on partitions) ----
                def qkv_matmul_half(w_sb, bias_off, n):
                    ps = qkv_ps.tile([128, 512], F32, tag="qkvps", name="qkvps")
                    sl = slice(n * 512, (n + 1) * 512)
                    nc.tensor.matmul(ps, lhsT=ones_sb,
                                     rhs=bias_sb[:, bias_off + n * 512: bias_off + (n + 1) * 512],
                                     start=True, stop=False)
                    for ki in range(8):
                        nc.tensor.matmul(ps, lhsT=hT[:, ki], rhs=w_sb[:, ki, sl],
                                         start=False, stop=(ki == 7))
                    return ps

                q_ps = [qkv_matmul_half(wq_sb, 0, n) for n in range(2)]
                k_ps = [qkv_matmul_half(wk_sb, IN_DIM, n) for n in range(2)]
                v_ps = [qkv_matmul_half(wv_sb, 2 * IN_DIM, n) for n in range(2)]

                # ---- feature map: elu(x)+1 = min(exp(x),1) + relu(x), then /(sum+eps)
                def elu_norm(ps_halves, tag):
                    nrm = work.tile([128, NUM_HEAD, DIM_HEAD], BF16, tag="n" + tag)
                    for n, ps in enumerate(ps_halves):
                        hh = slice(8 * n, 8 * n + 8)  # heads in this half
                        ex = work.tile([128, 512], F32, tag="ex", name="ex")
                        nc.scalar.activation(out=ex, in_=ps, func=AF.Exp)
                        rl = work.tile([128, 512], F32, tag="rl", name="rl")
                        nc.scalar.activation(out=rl, in_=ps, func=AF.Relu)
                        el = work.tile([128, 8, DIM_HEAD], F32, tag="el", name="el")
                        nc.vector.scalar_tensor_tensor(out=el.rearrange("p h d -> p (h d)"),
                                                       in0=ex, scalar=1.0,
                                                       in1=rl, op0=ALU.min, op1=ALU.add)
                        sm = small.tile([128, 8], F32, tag="s" + tag, name="sm")
                        nc.vector.tensor_reduce(out=sm, in_=el, axis=mybir.AxisListType.X, op=ALU.add)
                        nc.vector.tensor_scalar_add(out=sm, in0=sm, scalar1=1e-5)
                        nc.vector.reciprocal(out=sm, in_=sm)
                        nc.vector.tensor_tensor(out=nrm[:, hh], in0=el,
                                                in1=sm[:, :, None].to_broadcast(el.shape),
                                                op=ALU.mult)
                    return nrm

                qn = elu_norm(q_ps, "q")
                kn = elu_norm(k_ps, "k")
                vn = work.tile([128, NUM_HEAD, DIM_HEAD], BF16, tag="vn")
                for n in range(2):
                    nc.scalar.copy(out=vn[:, 8 * n: 8 * n + 8].rearrange("p h d -> p (h d)"),
                                   in_=v_ps[n])

                # ---- transpose q,k (head-dim on partitions) ----
                qnT = work.tile([128, NPAIR, 128], BF16, tag="qnT")
                knT = work.tile([128, NPAIR, 128], BF16, tag="knT")
                pe_transpose(qnT, qn.rearrange("p h d -> p (h d)"), 8)
                pe_transpose(knT, kn.rearrange("p h d -> p (h d)"), 8)

                # ---- pair-batched chunked attention (2 heads per tile) ----
                oT = work.tile([128, NPAIR, 128], BF16, tag="oT")
                for p in range(NPAIR):
                    h0, h1 = 2 * p, 2 * p + 1
                    lo, hi = slice(0, 64), slice(64, 128)
                    at = attn_ps.tile([128, 8, 128], F32, tag="at")
                    # concurrent row-tiled MMs must write DIFFERENT PSUM banks:
                    # h0 targets bank 0 (slots 0-3), h1 targets bank 1 (slots 4-7)
                    nc.tensor.matmul(at[:, 0], lhsT=knT[lo, p], rhs=qnT[lo, p],
                                     start=True, stop=True)
                    nc.tensor.matmul(at[:, 4], lhsT=knT[hi, p], rhs=qnT[hi, p],
                                     start=True, stop=True)
                    am = ampool.tile([128, 2, 128], BF16, tag="am")
                    nc.vector.tensor_tensor(out=am[:, 0], in0=at[:, 0], in1=mask_sb, op=ALU.mult)
                    nc.vector.tensor_tensor(out=am[:, 1], in0=at[:, 4], in1=mask_sb, op=ALU.mult)
                    nc.tensor.matmul(at[lo, 1], lhsT=state_bf[lo, p, lo],
                                     rhs=qnT[lo, p], start=True, stop=False)
                    nc.tensor.matmul(at[lo, 1], lhsT=vn[:, h0], rhs=am[:, 0],
                                     start=False, stop=True)
                    nc.tensor.matmul(at[lo, 5], lhsT=state_bf[hi, p, hi],
                                     rhs=qnT[hi, p], start=True, stop=False)
                    nc.tensor.matmul(at[lo, 5], lhsT=vn[:, h1], rhs=am[:, 1],
                                     start=False, stop=True)
                    nc.scalar.copy(out=oT[lo, p], in_=at[lo, 1])
                    nc.scalar.copy(out=oT[hi, p], in_=at[lo, 5])
                    # state update pair (off-diagonal cross terms are garbage,
                    # never read back: O_inter uses only diagonal blocks)
                    nc.tensor.matmul(at[:, 2], lhsT=kn[:, h0:h0 + 2], rhs=vn[:, h0:h0 + 2],
                                     start=True, stop=True)
                    nc.vector.tensor_add(out=state_sb[:, p], in0=state_sb[:, p], in1=at[:, 2])
                    nc.scalar.copy(out=state_bf[:, p], in_=state_sb[:, p])

                # ---- out matmul + residual ----
                for n in range(2):
                    sl = slice(n * 512, (n + 1) * 512)
                    ops = qkv_ps.tile([128, 512], F32, tag="qkvps", name="ops")
                    for p8 in range(8):
                        nc.tensor.matmul(ops, lhsT=oT[:, p8], rhs=wo_sb[:, p8, sl],
                                         start=(p8 == 0), stop=(p8 == 7))
                    nc.vector.tensor_add(out=y_big[:, c, sl], in0=ops, in1=x_t[:, sl])

            # ---- final outputs ----
            nc.gpsimd.dma_start(out=y_out.rearrange("(n c) i -> c n i", c=C), in_=y_big)
            nc.gpsimd.dma_start(out=wf_out[:], in_=state_sb)

    if split_waits:
        if not os.environ.get('NO_PRUNE'):
            _prune_implied_waits(nc)
        _split_excess_waits(nc)
    return nc


# ---------------- host side ----------------

def _prep_weights(W_qkv, W_out, ln_gamma, ln_beta):
    H, D3 = NUM_HEAD, 3 * DIM_HEAD
    W_eff = (W_qkv * ln_gamma[None, :]).astype(np.float32)
    bias_full = W_qkv.astype(np.float64) @ ln_beta.astype(np.float64)  # [3072]
    Wr = W_eff.reshape(H, D3, IN_DIM)
    bq = bias_full.reshape(H, D3)
    wq = Wr[:, 0:DIM_HEAD].reshape(H * DIM_HEAD, IN_DIM)
    wk = Wr[:, DIM_HEAD:2 * DIM_HEAD].reshape(H * DIM_HEAD, IN_DIM)
    wv = Wr[:, 2 * DIM_HEAD:].reshape(H * DIM_HEAD, IN_DIM)
    bias = np.concatenate([bq[:, 0:DIM_HEAD].reshape(-1),
                           bq[:, DIM_HEAD:2 * DIM_HEAD].reshape(-1),
                           bq[:, 2 * DIM_HEAD:].reshape(-1)])[None, :]
    bf = ml_dtypes.bfloat16
    return (np.ascontiguousarray(wq.T).astype(bf),
            np.ascontiguousarray(wk.T).astype(bf),
            np.ascontiguousarray(wv.T).astype(bf),
            np.ascontiguousarray(W_out.T).astype(bf),
            bias.astype(bf))


def _pack_state(state_b):
    out = np.zeros((128, NPAIR, 128), np.float32)
    for p in range(NPAIR):
        out[0:64, p, 0:64] = state_b[2 * p]
        out[64:128, p, 64:128] = state_b[2 * p + 1]
    return out


def _unpack_state(wf_dev):
    out = np.empty((NUM_HEAD, DIM_HEAD, DIM_HEAD), np.float32)
    for p in range(NPAIR):
        out[2 * p] = wf_dev[0:64, p, 0:64]
        out[2 * p + 1] = wf_dev[64:128, p, 64:128]
    return out


_NC_CACHE = {}
TRACE = False
LAST_RESULT = None


def kernel(x, state, W_qkv, W_out, ln_gamma, ln_beta):
    x = np.asarray(x, np.float32)
    state = np.asarray(state, np.float32)
    S, B, _ = x.shape
    n_chunks = S // C
    if n_chunks not in _NC_CACHE:
        _NC_CACHE[n_chunks] = build_nc(n_chunks)
    nc = _NC_CACHE[n_chunks]

    wq, wk, wv, wo, bias = _prep_weights(np.asarray(W_qkv, np.float32),
                                         np.asarray(W_out, np.float32),
                                         np.asarray(ln_gamma, np.float32),
                                         np.asarray(ln_beta, np.float32))
    mask = np.triu(np.ones((128, 128), np.float32))

    in_maps = []
    for b in range(B):
        in_maps.append({
            "x": np.ascontiguousarray(x[:, b, :]),
            "st": _pack_state(state[b]),
            "wq": wq, "wk": wk, "wv": wv, "wo": wo,
            "bqkv": bias, "mask": mask,
        })
    global LAST_RESULT
    res = run_bass_kernel_spmd(nc, in_maps, core_ids=list(range(B)), trace=TRACE)
    LAST_RESULT = res

    y = np.empty((S, B, IN_DIM), np.float32)
    wf = np.empty((B, NUM_HEAD, DIM_HEAD, DIM_HEAD), np.float32)
    for b in range(B):
        y[:, b, :] = res.results[b]["y"]
        wf[b] = _unpack_state(res.results[b]["wf"])
    return y, wf
